# revision 9
# baseline (speedup 1.0000x reference)
"""Bass/Trainium2 kernel for nn_DiagWeightLayer: out = x * weight (column scale).

x: (32768, 1024) f32, weight: (1024,) f32.
Data-parallel over 8 NeuronCores: each core processes a (4096, 1024) row
shard of x; the weight vector is replicated to every core.
"""

import numpy as np

import concourse.bacc as bacc
import concourse.bass as bass
import concourse.tile as tile
from concourse import mybir
from concourse.bass_utils import run_bass_kernel_spmd

N_CORES = 8
ROWS, COLS = 32768, 1024
SHARD = ROWS // N_CORES  # 4096 rows per core
P = 128  # SBUF partitions
N_CHUNKS = SHARD // P  # 32 row-chunks of 128 rows


def build(reps=1, blk=8, bufs=4, fused_mul=True, layout="pn", wmode="pbcast"):
    """Build the per-core Bass program.

    reps: repeat the whole compute (for wall-clock slope timing).
    blk: 128-row chunks packed per SBUF tile (one DMA each way per tile).
    bufs: tile-pool slots (pipeline depth).
    fused_mul: one tensor_mul per tile with free-dim-broadcast weight
        instead of one tensor_mul per chunk.
    layout: "pn" = consecutive rows per partition (contiguous blk*4KB DMA
        descriptor per partition line); "np" = round-robin rows across
        partitions (4KB descriptors).
    wmode: "pbcast" = 4KB DMA + gpsimd partition_broadcast; "dma" =
        128-descriptor broadcast DMA straight from DRAM.
    """
    assert N_CHUNKS % blk == 0
    n_tiles = N_CHUNKS // blk
    nc = bacc.Bacc()
    x = nc.dram_tensor("x", [SHARD, COLS], mybir.dt.float32, kind="ExternalInput")
    w = nc.dram_tensor("weight", [COLS], mybir.dt.float32, kind="ExternalInput")
    out = nc.dram_tensor("out", [SHARD, COLS], mybir.dt.float32, kind="ExternalOutput")

    # DRAM view: [partition, chunk, col].
    if layout == "pn":
        xv = x.rearrange("(p n) m -> p n m", p=P)
        ov = out.rearrange("(p n) m -> p n m", p=P)
    else:
        xv = x.rearrange("(n p) m -> p n m", p=P)
        ov = out.rearrange("(n p) m -> p n m", p=P)

    with tile.TileContext(nc) as tc:
        with (
            tc.tile_pool(name="singles", bufs=1) as singles,
            tc.tile_pool(name="xs", bufs=bufs) as xpool,
        ):
            # Replicate weight across all 128 partitions.
            w_sb = singles.tile([P, COLS], mybir.dt.float32)
            if wmode == "pbcast":
                nc.sync.dma_start(out=w_sb[:1, :], in_=w[None, :])
                nc.gpsimd.partition_broadcast(w_sb[:], w_sb[:1, :])
            else:
                nc.sync.dma_start(
                    out=w_sb[:], in_=w[None, :].to_broadcast([P, COLS])
                )

            for _ in range(reps):
                for i in range(n_tiles):
                    xt = xpool.tile([P, blk, COLS], mybir.dt.float32)
                    nc.sync.dma_start(
                        out=xt[:], in_=xv[:, i * blk : (i + 1) * blk, :]
                    )
                    if fused_mul:
                        nc.vector.tensor_mul(
                            xt[:], xt[:], w_sb[:, None, :].to_broadcast([P, blk, COLS])
                        )
                    else:
                        for j in range(blk):
                            nc.vector.tensor_mul(xt[:, j, :], xt[:, j, :], w_sb[:])
                    nc.sync.dma_start(
                        out=ov[:, i * blk : (i + 1) * blk, :], in_=xt[:]
                    )
    nc.finalize()
    return nc


_nc_cache = None


def _get_nc():
    global _nc_cache
    if _nc_cache is None:
        _nc_cache = build()
    return _nc_cache


def kernel(x: np.ndarray, weight: np.ndarray) -> np.ndarray:
    x = np.ascontiguousarray(x, dtype=np.float32)
    weight = np.ascontiguousarray(weight, dtype=np.float32)
    nc = _get_nc()
    in_maps = [
        {"x": x[i * SHARD : (i + 1) * SHARD], "weight": weight}
        for i in range(N_CORES)
    ]
    res = run_bass_kernel_spmd(nc, in_maps, list(range(N_CORES))).results
    return np.concatenate([r["out"] for r in res], axis=0)


# revision 17
# speedup vs baseline: 1.0424x; 1.0424x over previous
"""Bass/Trainium2 kernel for nn_DiagWeightLayer: out = x * weight (column scale).

x: (32768, 1024) f32, weight: (1024,) f32.
Data-parallel over 8 NeuronCores: each core processes a (4096, 1024) row
shard of x; the weight vector is replicated to every core.
"""

import numpy as np

import concourse.bacc as bacc
import concourse.bass as bass
import concourse.tile as tile
from concourse import mybir
from concourse.bass_utils import run_bass_kernel_spmd

N_CORES = 8
ROWS, COLS = 32768, 1024
SHARD = ROWS // N_CORES  # 4096 rows per core
P = 128  # SBUF partitions
N_CHUNKS = SHARD // P  # 32 row-chunks of 128 rows


def build(reps=1, blk=8, bufs=4, fused_mul=True, layout="pn", wmode="pbcast"):
    """Build the per-core Bass program.

    reps: repeat the whole compute (for wall-clock slope timing).
    blk: 128-row chunks packed per SBUF tile (one DMA each way per tile).
    bufs: tile-pool slots (pipeline depth).
    fused_mul: one tensor_mul per tile with free-dim-broadcast weight
        instead of one tensor_mul per chunk.
    layout: "pn" = consecutive rows per partition (contiguous blk*4KB DMA
        descriptor per partition line); "np" = round-robin rows across
        partitions (4KB descriptors).
    wmode: "pbcast" = 4KB DMA + gpsimd partition_broadcast; "dma" =
        128-descriptor broadcast DMA straight from DRAM.
    """
    assert N_CHUNKS % blk == 0
    n_tiles = N_CHUNKS // blk
    nc = bacc.Bacc()
    x = nc.dram_tensor("x", [SHARD, COLS], mybir.dt.float32, kind="ExternalInput")
    w = nc.dram_tensor("weight", [COLS], mybir.dt.float32, kind="ExternalInput")
    out = nc.dram_tensor("out", [SHARD, COLS], mybir.dt.float32, kind="ExternalOutput")

    # DRAM view: [partition, chunk, col].
    if layout == "pn":
        xv = x.rearrange("(p n) m -> p n m", p=P)
        ov = out.rearrange("(p n) m -> p n m", p=P)
    else:
        xv = x.rearrange("(n p) m -> p n m", p=P)
        ov = out.rearrange("(n p) m -> p n m", p=P)

    with tile.TileContext(nc) as tc:
        with (
            tc.tile_pool(name="singles", bufs=1) as singles,
            tc.tile_pool(name="xs", bufs=bufs) as xpool,
        ):
            # Replicate weight across all 128 partitions.
            w_sb = singles.tile([P, COLS], mybir.dt.float32)
            if wmode == "pbcast":
                nc.sync.dma_start(out=w_sb[:1, :], in_=w[None, :])
                nc.gpsimd.partition_broadcast(w_sb[:], w_sb[:1, :])
            else:
                nc.sync.dma_start(
                    out=w_sb[:], in_=w[None, :].to_broadcast([P, COLS])
                )

            for _ in range(reps):
                for i in range(n_tiles):
                    xt = xpool.tile([P, blk, COLS], mybir.dt.float32)
                    nc.sync.dma_start(
                        out=xt[:], in_=xv[:, i * blk : (i + 1) * blk, :]
                    )
                    if fused_mul:
                        nc.vector.tensor_mul(
                            xt[:], xt[:], w_sb[:, None, :].to_broadcast([P, blk, COLS])
                        )
                    else:
                        for j in range(blk):
                            nc.vector.tensor_mul(xt[:, j, :], xt[:, j, :], w_sb[:])
                    nc.sync.dma_start(
                        out=ov[:, i * blk : (i + 1) * blk, :], in_=xt[:]
                    )
    nc.finalize()
    return nc


_nc_cache = None


def _get_nc():
    global _nc_cache
    if _nc_cache is None:
        _nc_cache = build()
    return _nc_cache


def kernel(x: np.ndarray, weight: np.ndarray) -> np.ndarray:
    x = np.ascontiguousarray(x, dtype=np.float32)
    weight = np.ascontiguousarray(weight, dtype=np.float32)
    nc = _get_nc()
    in_maps = [
        {"x": x[i * SHARD : (i + 1) * SHARD], "weight": weight}
        for i in range(N_CORES)
    ]
    res = run_bass_kernel_spmd(nc, in_maps, list(range(N_CORES))).results
    return np.concatenate([r["out"] for r in res], axis=0)
